# revision 7
# baseline (speedup 1.0000x reference)
"""Trainium2 Bass kernel for BSplineActivation (KAN-style activation).

Reference computation (G=3 grid points on [-1,1], NUM_CP=5, degree 4):
    t        = clip(x, -1, 1)
    grid_idx = t + 1                       # in [0, 2]
    y_spline = lerp of s[floor], s[ceil]   where s[g] = basis_values[g] @ control_points
    out      = base_weight * silu(x) + spline_weight * y_spline

Because G=3, y_spline is piecewise LINEAR in t with breakpoints {-1, 0, 1}:
    y_spline(t) = s1 + (s2-s1)*t   for t >= 0
                = s1 + (s1-s0)*t   for t <  0
so with A = sw*(s2-s1), B = sw*(s1-s0), c = sw*s1 (all host-computed scalars):
    out = bw*silu(x) + c + L(t),   L(t) = A*t (t>=0) else B*t
L(t) folds into ONE parametric-relu:  sign * Prelu(|A|*t, alpha=B/A).

Per-core device program (pure data parallel over the batch dim, 1 batch/core):
    t = tensor_scalar(x, max -1, min 1)          # DVE, 2x fp32 mode
    u = Silu(x)                                  # ACT (silu_and_others set)
    w = Prelu(scale_w * t, alpha)                # ACT (same table set)
    u = u * bw + c                               # DVE tensor_scalar, in-place
    o = u (+|-) w                                # DVE tensor_tensor
    (Silu, Prelu, Copy, Abs all live in one ACT table set -> one table load.)

Engine busy (per core, 1 batch = 16 MiB in + 16 MiB out):
    DMA 93 us (roofline), DVE ~70 us, ACT ~58 us -> memory bound.
Tiles are 2 row-blocks wide ([128, 2, 2048], one 2 MiB DMA each) with the
last two tiles single-block to shorten the pipeline drain. Measured on HW:
~99.5 us/sweep vs a 96.2 us pure-copy floor.
"""

import numpy as np

# Problem shape (hardcoded; kernel.py must be self-contained).
BATCH = 8
ROWS = 2048
COLS = 2048
P = 128  # SBUF partitions
ROW_BLOCKS = ROWS // P  # 16 row-blocks of [128, 2048] per core
# Row-blocks per tile; tapered tail shortens the pipeline drain.
SCHEDULE = [2, 2, 2, 2, 2, 2, 2, 1, 1]
BUFS_IO = 3   # x-in / out tiles: triple buffering
BUFS_MID = 2  # intermediate tiles


def _build_nc(bw, c, scale_w, alpha, sign, repeat=1):
    import concourse.bacc as bacc
    import concourse.mybir as mybir
    from concourse.tile import TileContext

    f32 = mybir.dt.float32
    AF = mybir.ActivationFunctionType
    ALU = mybir.AluOpType

    assert sum(SCHEDULE) == ROW_BLOCKS

    nc = bacc.Bacc("TRN2")
    x = nc.dram_tensor("x", [ROWS, COLS], f32, kind="ExternalInput")
    out = nc.dram_tensor("out", [ROWS, COLS], f32, kind="ExternalOutput")
    xv = x.rearrange("(a p) f -> a p f", p=P)     # [16, 128, 2048]
    ov = out.rearrange("(a p) f -> a p f", p=P)
    tt_op = ALU.add if sign > 0 else ALU.subtract

    def body(pio, pmid):
        s = 0
        for nb in SCHEDULE:
            shape = [P, nb, COLS] if nb > 1 else [P, COLS]
            src = (xv[s] if nb == 1
                   else xv[s:s + nb].rearrange("b p f -> p b f"))
            dst = (ov[s] if nb == 1
                   else ov[s:s + nb].rearrange("b p f -> p b f"))

            xt = pio.tile(shape, f32, tag="xt")
            nc.sync.dma_start(out=xt, in_=src)

            u = pmid.tile(shape, f32, tag="u")
            nc.scalar.activation(out=u, in_=xt, func=AF.Silu)

            t = pmid.tile(shape, f32, tag="t")
            nc.vector.tensor_scalar(out=t, in0=xt, scalar1=-1.0, scalar2=1.0,
                                    op0=ALU.max, op1=ALU.min)

            w = pmid.tile(shape, f32, tag="w")
            nc.scalar.activation(out=w, in_=t, func=AF.Prelu,
                                 scale=float(scale_w), alpha=float(alpha))

            # u <- u*bw + c (in-place on DVE)
            nc.vector.tensor_scalar(out=u, in0=u, scalar1=float(bw),
                                    scalar2=float(c), op0=ALU.mult,
                                    op1=ALU.add)

            o = pio.tile(shape, f32, tag="o")
            nc.vector.tensor_tensor(out=o, in0=u, in1=w, op=tt_op)

            nc.sync.dma_start(out=dst, in_=o)
            s += nb

    with TileContext(nc) as tc:
        with tc.tile_pool(name="pio", bufs=BUFS_IO) as pio, \
             tc.tile_pool(name="pmid", bufs=BUFS_MID) as pmid:
            if repeat == 1:
                body(pio, pmid)
            else:
                with tc.For_i(0, repeat, 1):
                    body(pio, pmid)

    nc.compile()
    return nc


def _host_constants(control_points, base_weight, spline_weight, basis_values):
    cp = np.asarray(control_points, dtype=np.float64)
    bv = np.asarray(basis_values, dtype=np.float64)
    bw = float(np.asarray(base_weight).reshape(-1)[0])
    sw = float(np.asarray(spline_weight).reshape(-1)[0])
    s = bv @ cp  # s[g] = dot(basis_values[g], control_points), g in {0,1,2}
    c = sw * s[1]
    A = sw * (s[2] - s[1])  # slope for t >= 0
    B = sw * (s[1] - s[0])  # slope for t < 0
    if A != 0.0:
        scale_w, alpha, sign = abs(A), B / A, (1.0 if A > 0 else -1.0)
    elif B != 0.0:
        scale_w, alpha, sign = (-B if B > 0 else B), 0.0, (-1.0 if B > 0 else 1.0)
    else:
        scale_w, alpha, sign = 0.0, 0.0, 1.0
    return bw, c, scale_w, alpha, sign


def kernel(x, control_points, base_weight, spline_weight, basis_values,
           _repeat=1, _return_nc=False):
    from concourse.bass_utils import run_bass_kernel_spmd

    x = np.ascontiguousarray(np.asarray(x, dtype=np.float32))
    assert x.shape == (BATCH, ROWS, COLS), x.shape

    bw, c, scale_w, alpha, sign = _host_constants(
        control_points, base_weight, spline_weight, basis_values
    )
    nc = _build_nc(bw, c, scale_w, alpha, sign, repeat=_repeat)
    if _return_nc:
        return nc

    in_maps = [{"x": x[i]} for i in range(BATCH)]
    res = run_bass_kernel_spmd(nc, in_maps, core_ids=list(range(BATCH)))
    out = np.stack([res.results[i]["out"] for i in range(BATCH)], axis=0)
    return out.astype(np.float32, copy=False)


# revision 9
# speedup vs baseline: 1.0597x; 1.0597x over previous
"""Trainium2 Bass kernel for BSplineActivation (KAN-style activation).

Reference computation (G=3 grid points on [-1,1], NUM_CP=5, degree 4):
    t        = clip(x, -1, 1)
    grid_idx = t + 1                       # in [0, 2]
    y_spline = lerp of s[floor], s[ceil]   where s[g] = basis_values[g] @ control_points
    out      = base_weight * silu(x) + spline_weight * y_spline

Because G=3, y_spline is piecewise LINEAR in t with breakpoints {-1, 0, 1}:
    y_spline(t) = s1 + (s2-s1)*t   for t >= 0
                = s1 + (s1-s0)*t   for t <  0
so with A = sw*(s2-s1), B = sw*(s1-s0), c = sw*s1 (all host-computed scalars):
    out = bw*silu(x) + c + L(t),   L(t) = A*t (t>=0) else B*t
L(t) folds into ONE parametric-relu:  sign * Prelu(|A|*t, alpha=B/A).

Per-core device program (pure data parallel over the batch dim, 1 batch/core):
    t = tensor_scalar(x, max -1, min 1)          # DVE, 2x fp32 mode
    u = Silu(x)                                  # ACT (silu_and_others set)
    w = Prelu(scale_w * t, alpha)                # ACT (same table set)
    u = u * bw + c                               # DVE tensor_scalar, in-place
    o = u (+|-) w                                # DVE tensor_tensor
    (Silu, Prelu, Copy, Abs all live in one ACT table set -> one table load.)

Engine busy (per core, 1 batch = 16 MiB in + 16 MiB out):
    DMA 93 us (roofline), DVE ~70 us, ACT ~58 us -> memory bound.
Tiles are 2 row-blocks wide ([128, 2, 2048], one 2 MiB DMA each) with the
last two tiles single-block to shorten the pipeline drain. Measured on HW:
~99.5 us/sweep vs a 96.2 us pure-copy floor.
"""

import numpy as np

# Problem shape (hardcoded; kernel.py must be self-contained).
BATCH = 8
ROWS = 2048
COLS = 2048
P = 128  # SBUF partitions
ROW_BLOCKS = ROWS // P  # 16 row-blocks of [128, 2048] per core
# Per-tile (row_blocks, mode). The tapered tail shortens the pipeline drain:
#  - 'whole':    one TT + one out-DMA for the whole tile
#  - 'rowsplit': TT + out-DMA per row-block (out starts after half the TT)
#  - 'colsplit4': full chain per column-quarter (shortest drain, last tile)
SCHEDULE = [(2, "whole")] * 6 + [(2, "rowsplit"), (1, "whole"), (1, "colsplit4")]
BUFS_IO = 3   # x-in / out tiles: triple buffering
BUFS_MID = 2  # intermediate tiles


def _build_nc(bw, c, scale_w, alpha, sign, repeat=1):
    import concourse.bacc as bacc
    import concourse.mybir as mybir
    from concourse.tile import TileContext

    f32 = mybir.dt.float32
    AF = mybir.ActivationFunctionType
    ALU = mybir.AluOpType

    assert sum(nb for nb, _ in SCHEDULE) == ROW_BLOCKS

    nc = bacc.Bacc("TRN2")
    x = nc.dram_tensor("x", [ROWS, COLS], f32, kind="ExternalInput")
    out = nc.dram_tensor("out", [ROWS, COLS], f32, kind="ExternalOutput")
    xv = x.rearrange("(a p) f -> a p f", p=P)     # [16, 128, 2048]
    ov = out.rearrange("(a p) f -> a p f", p=P)
    tt_op = ALU.add if sign > 0 else ALU.subtract

    def compute(xt, u, t, w, o):
        """Emit the 5-op elementwise chain on matching AP slices."""
        nc.scalar.activation(out=u, in_=xt, func=AF.Silu)
        nc.vector.tensor_scalar(out=t, in0=xt, scalar1=-1.0, scalar2=1.0,
                                op0=ALU.max, op1=ALU.min)
        nc.scalar.activation(out=w, in_=t, func=AF.Prelu,
                             scale=float(scale_w), alpha=float(alpha))
        # u <- u*bw + c (in-place on DVE)
        nc.vector.tensor_scalar(out=u, in0=u, scalar1=float(bw),
                                scalar2=float(c), op0=ALU.mult, op1=ALU.add)
        nc.vector.tensor_tensor(out=o, in0=u, in1=w, op=tt_op)

    def body(pio, pmid):
        s = 0
        for nb, mode in SCHEDULE:
            shape = [P, nb, COLS] if nb > 1 else [P, COLS]
            src = (xv[s] if nb == 1
                   else xv[s:s + nb].rearrange("b p f -> p b f"))

            xt = pio.tile(shape, f32, tag="xt")
            nc.sync.dma_start(out=xt, in_=src)
            u = pmid.tile(shape, f32, tag="u")
            t = pmid.tile(shape, f32, tag="t")
            w = pmid.tile(shape, f32, tag="w")
            o = pio.tile(shape, f32, tag="o")

            if mode == "colsplit4":
                assert nb == 1
                H = COLS // 4
                for h in range(4):
                    sl = slice(h * H, (h + 1) * H)
                    compute(xt[:, sl], u[:, sl], t[:, sl], w[:, sl], o[:, sl])
                    nc.sync.dma_start(out=ov[s][:, sl], in_=o[:, sl])
            elif mode == "rowsplit" and nb > 1:
                nc.scalar.activation(out=u, in_=xt, func=AF.Silu)
                nc.vector.tensor_scalar(out=t, in0=xt, scalar1=-1.0,
                                        scalar2=1.0, op0=ALU.max, op1=ALU.min)
                nc.scalar.activation(out=w, in_=t, func=AF.Prelu,
                                     scale=float(scale_w), alpha=float(alpha))
                nc.vector.tensor_scalar(out=u, in0=u, scalar1=float(bw),
                                        scalar2=float(c), op0=ALU.mult,
                                        op1=ALU.add)
                for b in range(nb):
                    nc.vector.tensor_tensor(out=o[:, b, :], in0=u[:, b, :],
                                            in1=w[:, b, :], op=tt_op)
                    nc.sync.dma_start(out=ov[s + b], in_=o[:, b, :])
            else:
                dst = (ov[s] if nb == 1
                       else ov[s:s + nb].rearrange("b p f -> p b f"))
                compute(xt, u, t, w, o)
                nc.sync.dma_start(out=dst, in_=o)
            s += nb

    with TileContext(nc) as tc:
        with tc.tile_pool(name="pio", bufs=BUFS_IO) as pio, \
             tc.tile_pool(name="pmid", bufs=BUFS_MID) as pmid:
            if repeat == 1:
                body(pio, pmid)
            else:
                with tc.For_i(0, repeat, 1):
                    body(pio, pmid)

    nc.compile()
    return nc


def _host_constants(control_points, base_weight, spline_weight, basis_values):
    cp = np.asarray(control_points, dtype=np.float64)
    bv = np.asarray(basis_values, dtype=np.float64)
    bw = float(np.asarray(base_weight).reshape(-1)[0])
    sw = float(np.asarray(spline_weight).reshape(-1)[0])
    s = bv @ cp  # s[g] = dot(basis_values[g], control_points), g in {0,1,2}
    c = sw * s[1]
    A = sw * (s[2] - s[1])  # slope for t >= 0
    B = sw * (s[1] - s[0])  # slope for t < 0
    if A != 0.0:
        scale_w, alpha, sign = abs(A), B / A, (1.0 if A > 0 else -1.0)
    elif B != 0.0:
        scale_w, alpha, sign = (-B if B > 0 else B), 0.0, (-1.0 if B > 0 else 1.0)
    else:
        scale_w, alpha, sign = 0.0, 0.0, 1.0
    return bw, c, scale_w, alpha, sign


def kernel(x, control_points, base_weight, spline_weight, basis_values,
           _repeat=1, _return_nc=False):
    from concourse.bass_utils import run_bass_kernel_spmd

    x = np.ascontiguousarray(np.asarray(x, dtype=np.float32))
    assert x.shape == (BATCH, ROWS, COLS), x.shape

    bw, c, scale_w, alpha, sign = _host_constants(
        control_points, base_weight, spline_weight, basis_values
    )
    nc = _build_nc(bw, c, scale_w, alpha, sign, repeat=_repeat)
    if _return_nc:
        return nc

    in_maps = [{"x": x[i]} for i in range(BATCH)]
    res = run_bass_kernel_spmd(nc, in_maps, core_ids=list(range(BATCH)))
    out = np.stack([res.results[i]["out"] for i in range(BATCH)], axis=0)
    return out.astype(np.float32, copy=False)


# revision 14
# speedup vs baseline: 1.0683x; 1.0082x over previous
"""Trainium2 Bass kernel for BSplineActivation (KAN-style activation).

Reference computation (G=3 grid points on [-1,1], NUM_CP=5, degree 4):
    t        = clip(x, -1, 1)
    grid_idx = t + 1                       # in [0, 2]
    y_spline = lerp of s[floor], s[ceil]   where s[g] = basis_values[g] @ control_points
    out      = base_weight * silu(x) + spline_weight * y_spline

Because G=3, y_spline is piecewise LINEAR in t with breakpoints {-1, 0, 1}:
    y_spline(t) = s1 + (s2-s1)*t   for t >= 0
                = s1 + (s1-s0)*t   for t <  0
so with A = sw*(s2-s1), B = sw*(s1-s0), c = sw*s1 (all host-computed scalars):
    out = bw*silu(x) + c + L(t),   L(t) = A*t (t>=0) else B*t
L(t) folds into ONE parametric-relu:  sign * Prelu(|A|*t, alpha=B/A).

Per-core device program (pure data parallel over the batch dim, 1 batch/core):
    t = tensor_scalar(x, max -1, min 1)          # DVE, 2x fp32 mode
    u = Silu(x)                                  # ACT (silu_and_others set)
    w = Prelu(scale_w * t, alpha)                # ACT (same table set)
    u = u * bw + c                               # DVE tensor_scalar, in-place
    o = u (+|-) w                                # DVE tensor_tensor
    (Silu, Prelu, Copy, Abs all live in one ACT table set -> one table load.)

Engine busy (per core, 1 batch = 16 MiB in + 16 MiB out):
    DMA 93.2 us (HBM roofline @360GB/s/core), DVE ~70 us, ACT ~58 us
    -> memory bound; all DMAs issued from nc.sync (one SP HWDGE queue
    saturates per-core HBM).
Tiles are 2 row-blocks wide ([128, 2, 2048], one 2 MiB DMA each), tapered
at the tail (row-split, then a column-quartered final tile) to shorten the
pipeline drain. TimelineSim one-shot: 97.2 us vs 96.6 us for a pure DMA
copy; measured HW sweep 96-99 us (K-repeat slope, vs 93.2 us copy body).
"""

import numpy as np

# Problem shape (hardcoded; kernel.py must be self-contained).
BATCH = 8
ROWS = 2048
COLS = 2048
P = 128  # SBUF partitions
ROW_BLOCKS = ROWS // P  # 16 row-blocks of [128, 2048] per core
# Per-tile (row_blocks, mode). The tapered tail shortens the pipeline drain:
#  - 'whole':    one TT + one out-DMA for the whole tile
#  - 'rowsplit': TT + out-DMA per row-block (out starts after half the TT)
#  - 'colsplit4': full chain per column-quarter (shortest drain, last tile)
SCHEDULE = [(2, "whole")] * 6 + [(2, "rowsplit"), (1, "whole"), (1, "colsplit4")]
BUFS_IO = 3   # x-in / out tiles: triple buffering
BUFS_MID = 2  # intermediate tiles


def _build_nc(bw, c, scale_w, alpha, sign, repeat=1):
    import concourse.bacc as bacc
    import concourse.mybir as mybir
    from concourse.tile import TileContext

    f32 = mybir.dt.float32
    AF = mybir.ActivationFunctionType
    ALU = mybir.AluOpType

    assert sum(nb for nb, _ in SCHEDULE) == ROW_BLOCKS

    nc = bacc.Bacc("TRN2")
    x = nc.dram_tensor("x", [ROWS, COLS], f32, kind="ExternalInput")
    out = nc.dram_tensor("out", [ROWS, COLS], f32, kind="ExternalOutput")
    xv = x.rearrange("(a p) f -> a p f", p=P)     # [16, 128, 2048]
    ov = out.rearrange("(a p) f -> a p f", p=P)
    tt_op = ALU.add if sign > 0 else ALU.subtract

    def combine(u, w, o):
        """o = bw*u + c + sign*w (affine in-place on u, then tensor_tensor).
        Note: the fused AFFINE_THEN_ADD custom DVE op computes this in one
        pass but measured SLOWER end-to-end (100.4 vs 96.4 us/sweep on HW;
        mixing builtin and custom-table DVE ops thrashes the uop library)."""
        nc.vector.tensor_scalar(out=u, in0=u, scalar1=float(bw),
                                scalar2=float(c), op0=ALU.mult, op1=ALU.add)
        nc.vector.tensor_tensor(out=o, in0=u, in1=w, op=tt_op)

    def compute(xt, u, t, w, o):
        """Emit the elementwise chain on matching AP slices."""
        nc.scalar.activation(out=u, in_=xt, func=AF.Silu)
        nc.vector.tensor_scalar(out=t, in0=xt, scalar1=-1.0, scalar2=1.0,
                                op0=ALU.max, op1=ALU.min)
        nc.scalar.activation(out=w, in_=t, func=AF.Prelu,
                             scale=float(scale_w), alpha=float(alpha))
        combine(u, w, o)

    def body(pio, pmid):
        s = 0
        for nb, mode in SCHEDULE:
            shape = [P, nb, COLS] if nb > 1 else [P, COLS]
            src = (xv[s] if nb == 1
                   else xv[s:s + nb].rearrange("b p f -> p b f"))

            xt = pio.tile(shape, f32, tag="xt")
            nc.sync.dma_start(out=xt, in_=src)
            u = pmid.tile(shape, f32, tag="u")
            t = pmid.tile(shape, f32, tag="t")
            w = pmid.tile(shape, f32, tag="w")
            o = pio.tile(shape, f32, tag="o")

            if mode == "colsplit4":
                assert nb == 1
                H = COLS // 4
                for h in range(4):
                    sl = slice(h * H, (h + 1) * H)
                    compute(xt[:, sl], u[:, sl], t[:, sl], w[:, sl], o[:, sl])
                    nc.sync.dma_start(out=ov[s][:, sl], in_=o[:, sl])
            elif mode == "rowsplit" and nb > 1:
                nc.scalar.activation(out=u, in_=xt, func=AF.Silu)
                nc.vector.tensor_scalar(out=t, in0=xt, scalar1=-1.0,
                                        scalar2=1.0, op0=ALU.max, op1=ALU.min)
                nc.scalar.activation(out=w, in_=t, func=AF.Prelu,
                                     scale=float(scale_w), alpha=float(alpha))
                nc.vector.tensor_scalar(out=u, in0=u, scalar1=float(bw),
                                        scalar2=float(c), op0=ALU.mult,
                                        op1=ALU.add)
                for b in range(nb):
                    nc.vector.tensor_tensor(out=o[:, b, :], in0=u[:, b, :],
                                            in1=w[:, b, :], op=tt_op)
                    nc.sync.dma_start(out=ov[s + b], in_=o[:, b, :])
            else:
                dst = (ov[s] if nb == 1
                       else ov[s:s + nb].rearrange("b p f -> p b f"))
                compute(xt, u, t, w, o)
                nc.sync.dma_start(out=dst, in_=o)
            s += nb

    with TileContext(nc) as tc:
        with tc.tile_pool(name="pio", bufs=BUFS_IO) as pio, \
             tc.tile_pool(name="pmid", bufs=BUFS_MID) as pmid:
            if repeat == 1:
                body(pio, pmid)
            else:
                with tc.For_i(0, repeat, 1):
                    body(pio, pmid)

    nc.compile()
    return nc


def _host_constants(control_points, base_weight, spline_weight, basis_values):
    cp = np.asarray(control_points, dtype=np.float64)
    bv = np.asarray(basis_values, dtype=np.float64)
    bw = float(np.asarray(base_weight).reshape(-1)[0])
    sw = float(np.asarray(spline_weight).reshape(-1)[0])
    s = bv @ cp  # s[g] = dot(basis_values[g], control_points), g in {0,1,2}
    c = sw * s[1]
    A = sw * (s[2] - s[1])  # slope for t >= 0
    B = sw * (s[1] - s[0])  # slope for t < 0
    if A != 0.0:
        scale_w, alpha, sign = abs(A), B / A, (1.0 if A > 0 else -1.0)
    elif B != 0.0:
        scale_w, alpha, sign = (-B if B > 0 else B), 0.0, (-1.0 if B > 0 else 1.0)
    else:
        scale_w, alpha, sign = 0.0, 0.0, 1.0
    return bw, c, scale_w, alpha, sign


def kernel(x, control_points, base_weight, spline_weight, basis_values,
           _repeat=1, _return_nc=False):
    from concourse.bass_utils import run_bass_kernel_spmd

    x = np.ascontiguousarray(np.asarray(x, dtype=np.float32))
    assert x.shape == (BATCH, ROWS, COLS), x.shape

    bw, c, scale_w, alpha, sign = _host_constants(
        control_points, base_weight, spline_weight, basis_values
    )
    nc = _build_nc(bw, c, scale_w, alpha, sign, repeat=_repeat)
    if _return_nc:
        return nc

    in_maps = [{"x": x[i]} for i in range(BATCH)]
    res = run_bass_kernel_spmd(nc, in_maps, core_ids=list(range(BATCH)))
    out = np.stack([res.results[i]["out"] for i in range(BATCH)], axis=0)
    return out.astype(np.float32, copy=False)
